# revision 3
# baseline (speedup 1.0000x reference)
"""Trainium2 Bass kernel for a single-head causal attention block.

Problem (hardcoded):
  input_val: [4, 4096, 1024] f32, Wq/Wk/Wv: [64, 1024] f32, k_mask: [4, 4096] i32
  out = softmax(causal_mask(QK^T/sqrt(64))) @ V  -> [4, 4096, 64] f32

Sharding: 8 cores = 4 batches x 2 roles. Within a batch, queries are split
into 16 chunks of 256; role r takes chunks {2j+r}. Every core runs an
IDENTICAL program (SPMD); the chunk identity is carried entirely by the
per-core input data (xq rows, causal-mask tile contents).

Device program per core (all matmul operands in SBUF partitions 0:127,
contraction bases at partition 0 - base-64 operands crash this runtime):
  - K^T/V^T projection: lhsT = [Wk^T|Wv^T] (c-chunked), rhs = X^T -> PSUM
    [128=K^T(64)|V^T(64), 512] per t-block, copied to bf16 SBUF.
  - V^T -> V via DMA xbar transpose, packed as V1[kb] = [V | ones] (65 cols)
  - Q^T projection from xq (own queries, locally contiguous)
  - attention, flash style in S^T layout: for local chunk j (256 queries),
    k-block groups g=0..j (4 kb of 128 keys each):
      S^T psum [128, 4, 256] = K^T_kb.T @ Q^T_chunk  (K=64 contraction)
      P^T = exp(S^T / 8) via one ACT activation (N=1024)
      last group: P^T *= causal mask tile (DVE)
      O^T psum [65, 256] += V1[kb].T @ P^T  (fused numerator + denominator)
  - out [65, 2048] f32 = per-chunk O^T; host divides by row 64 and scatters.
"""
import numpy as np

B, T, C, H = 4, 4096, 1024, 64
N_CORES = 8
CC = 8          # contraction chunks of 128 over C
TB = 8          # key t-blocks of 512
QB = 4          # local query blocks of 512 (for Q projection)
NKB = 32        # key blocks of 128
NCH = 8         # local query chunks of 256
SCALE = 1.0 / np.sqrt(H)

_CACHE = {}


def _build_program(use_kmask: bool):
    from contextlib import ExitStack
    import concourse.tile as tile
    from concourse import bacc, mybir

    BF16 = mybir.dt.bfloat16
    F32 = mybir.dt.float32
    Exp = mybir.ActivationFunctionType.Exp

    nc = bacc.Bacc("TRN2", target_bir_lowering=False, debug=False,
                   num_devices=N_CORES)
    xt = nc.dram_tensor("xt", [TB, 128, CC, 512], BF16, kind="ExternalInput")
    xq = nc.dram_tensor("xq", [QB, 128, CC, 512], BF16, kind="ExternalInput")
    wkv = nc.dram_tensor("wkv", [128, CC, 128], BF16, kind="ExternalInput")
    wq = nc.dram_tensor("wq", [128, CC, 64], BF16, kind="ExternalInput")
    cm = nc.dram_tensor("cm", [128, 4, 256], BF16, kind="ExternalInput")
    if use_kmask:
        km = nc.dram_tensor("km", [128, NKB], BF16, kind="ExternalInput")
    o = nc.dram_tensor("o", [65, 2048], F32, kind="ExternalOutput")

    with tile.TileContext(nc) as tc:
        with ExitStack() as ctx:
            const = ctx.enter_context(tc.tile_pool(name="const", bufs=1))
            ppool = ctx.enter_context(tc.tile_pool(name="pt", bufs=3))
            projp = ctx.enter_context(tc.tile_pool(name="projp", bufs=2, space="PSUM"))
            stp = ctx.enter_context(tc.tile_pool(name="stp", bufs=2, space="PSUM"))
            otp = ctx.enter_context(tc.tile_pool(name="otp", bufs=2, space="PSUM"))

            XT = const.tile([128, TB, CC, 512], BF16)
            XQ = const.tile([128, QB, CC, 512], BF16)
            WKV = const.tile([128, CC, 128], BF16)
            WQ = const.tile([128, CC, 64], BF16)
            CM = const.tile([128, 4, 256], BF16)
            KT = const.tile([64, T], BF16)
            QT = const.tile([64, 2048], BF16)
            VT = const.tile([128, T], BF16)       # rows 64:128 hold V^T
            V1 = const.tile([128, NKB, 80], BF16)  # [:, kb, 0:64]=V, 64 = ones
            OUT = const.tile([65, 2048], F32)
            if use_kmask:
                KM = const.tile([128, NKB], BF16)
                nc.sync.dma_start(KM[:], km.ap())

            nc.sync.dma_start(WKV[:], wkv.ap())
            nc.sync.dma_start(WQ[:], wq.ap())
            nc.sync.dma_start(CM[:], cm.ap())
            nc.gpsimd.memset(V1[:, :, 64:65], 1.0)

            def emit_kv(tb):
                nc.sync.dma_start(XT[:, tb], xt.ap()[tb])
                ps = projp.tile([128, 512], F32)
                for cc in range(CC):
                    nc.tensor.matmul(ps[:], WKV[:, cc, :], XT[:, tb, cc, :],
                                     start=(cc == 0), stop=(cc == CC - 1))
                sl = slice(512 * tb, 512 * (tb + 1))
                nc.vector.tensor_copy(KT[:, sl], ps[0:64, :])
                nc.vector.tensor_copy(VT[64:128, sl], ps[64:128, :])
                for kb in range(4 * tb, 4 * tb + 4):
                    nc.sync.dma_start_transpose(
                        V1[:, kb, 0:64], VT[64:128, 128 * kb:128 * (kb + 1)])

            def emit_q(qb):
                nc.sync.dma_start(XQ[:, qb], xq.ap()[qb])
                ps = projp.tile([128, 512], F32)
                for cc in range(CC):
                    nc.tensor.matmul(ps[0:64, :], WQ[:, cc, :], XQ[:, qb, cc, :],
                                     start=(cc == 0), stop=(cc == CC - 1))
                nc.vector.tensor_copy(QT[:, 512 * qb:512 * (qb + 1)], ps[0:64, :])

            def emit_S(j, g):
                st = stp.tile([128, 4, 256], F32)
                for u in range(4):
                    kb = 4 * g + u
                    nc.tensor.matmul(st[:, u, :], KT[:, 128 * kb:128 * (kb + 1)],
                                     QT[:, 256 * j:256 * (j + 1)],
                                     start=True, stop=True)
                pt = ppool.tile([128, 4, 256], BF16)
                nc.scalar.activation(pt[:], st[:], Exp, scale=SCALE)
                if g == j:
                    nc.vector.tensor_mul(pt[:], pt[:], CM[:])
                if use_kmask:
                    for u in range(4):
                        kb = 4 * g + u
                        nc.vector.tensor_scalar_mul(pt[:, u, :], pt[:, u, :],
                                                    KM[:, kb:kb + 1])
                return pt

            def emit_O(j, g, pt, oacc):
                for u in range(4):
                    kb = 4 * g + u
                    nc.tensor.matmul(oacc[:], V1[:, kb, 0:65], pt[:, u, :],
                                     start=(kb == 0), stop=(kb == 4 * j + 3))
                if g == j:
                    nc.vector.tensor_copy(OUT[:, 256 * j:256 * (j + 1)], oacc[:])

            pending = None
            oaccs = {}
            for j in range(NCH):
                emit_kv(j)
                if j % 2 == 0:
                    emit_q(j // 2)
                oaccs[j] = otp.tile([65, 256], F32, name="oacc", tag="oacc")
                for g in range(j + 1):
                    pt = emit_S(j, g)
                    if pending is not None:
                        emit_O(pending[0], pending[1], pending[2], oaccs[pending[0]])
                    pending = (j, g, pt)
            emit_O(pending[0], pending[1], pending[2], oaccs[pending[0]])

            nc.sync.dma_start(o.ap(), OUT[:])

    nc.compile()
    return nc


def _get_program(use_kmask: bool):
    key = ("prog", use_kmask)
    if key not in _CACHE:
        _CACHE[key] = _build_program(use_kmask)
    return _CACHE[key]


def _host_prep(input_val, Wq, Wk, Wv, k_mask, use_kmask):
    import ml_dtypes
    bf = ml_dtypes.bfloat16

    wkv_np = np.empty((128, CC, 128), dtype=bf)
    wkv_np[:, :, 0:64] = Wk.reshape(64, CC, 128).transpose(2, 1, 0).astype(bf)
    wkv_np[:, :, 64:128] = Wv.reshape(64, CC, 128).transpose(2, 1, 0).astype(bf)
    wq_np = Wq.reshape(64, CC, 128).transpose(2, 1, 0).astype(bf).copy()

    kk = np.arange(128)[:, None]
    qq = np.arange(256)[None, :]
    cms = []
    for r in range(2):
        cm_np = np.empty((128, 4, 256), dtype=bf)
        for t in range(4):
            cm_np[:, t, :] = (kk <= 256 * r - 128 * t + qq).astype(bf)
        cms.append(cm_np)

    xts = []
    for b in range(B):
        Xb = np.asarray(input_val[b], dtype=np.float32)
        xts.append(np.ascontiguousarray(
            Xb.reshape(TB, 512, CC, 128).transpose(0, 3, 2, 1)).astype(bf))

    in_maps = []
    for c in range(N_CORES):
        b, r = c // 2, c % 2
        Xb = np.asarray(input_val[b], dtype=np.float32)
        rows = np.concatenate(
            [np.arange(256 * (2 * j + r), 256 * (2 * j + r) + 256)
             for j in range(NCH)])
        Xqrows = Xb[rows]  # [2048, 1024]
        xq_np = np.ascontiguousarray(
            Xqrows.reshape(QB, 512, CC, 128).transpose(0, 3, 2, 1)).astype(bf)
        m = {"xt": xts[b], "xq": xq_np, "wkv": wkv_np, "wq": wq_np,
             "cm": cms[r]}
        if use_kmask:
            m["km"] = np.asarray(k_mask[b], dtype=np.float32).reshape(
                NKB, 128).T.astype(bf).copy()
        in_maps.append(m)
    return in_maps


def _unshard(results):
    out = np.empty((B, T, H), dtype=np.float32)
    for c in range(N_CORES):
        b, r = c // 2, c % 2
        on = results[c]["o"]
        num = on[0:64, :]
        den = on[64, :]
        for j in range(NCH):
            g = 2 * j + r
            blk = num[:, 256 * j:256 * (j + 1)] / den[None, 256 * j:256 * (j + 1)]
            out[b, 256 * g:256 * (g + 1), :] = blk.T
    return out


def kernel(input_val, Wq, Wk, Wv, k_mask):
    import concourse.bass_utils as bu

    input_val = np.asarray(input_val)
    Wq, Wk, Wv = (np.asarray(a, dtype=np.float32) for a in (Wq, Wk, Wv))
    k_mask = np.asarray(k_mask)
    use_kmask = not bool(np.all(k_mask == 1))

    nc = _get_program(use_kmask)
    in_maps = _host_prep(input_val, Wq, Wk, Wv, k_mask, use_kmask)
    res = bu.run_bass_kernel_spmd(nc, in_maps, core_ids=list(range(N_CORES)))
    return _unshard(res.results)


def kernel_traced(input_val, Wq, Wk, Wv, k_mask, **trace_kwargs):
    """Like kernel() but returns (output, BassKernelResults) with tracing on.

    Used by test.py for HW timing; requires the antenv.axon_hooks shim (the
    caller sets it up)."""
    import concourse.bass_utils as bu

    input_val = np.asarray(input_val)
    k_mask = np.asarray(k_mask)
    use_kmask = not bool(np.all(k_mask == 1))
    nc = _get_program(use_kmask)
    in_maps = _host_prep(input_val, np.asarray(Wq, dtype=np.float32),
                         np.asarray(Wk, dtype=np.float32),
                         np.asarray(Wv, dtype=np.float32), k_mask, use_kmask)
    res = bu.run_bass_kernel_spmd(nc, in_maps, core_ids=list(range(N_CORES)),
                                  trace=True, **trace_kwargs)
    return _unshard(res.results), res


# revision 4
# speedup vs baseline: 1.0158x; 1.0158x over previous
"""Trainium2 Bass kernel for a single-head causal attention block.

Problem (hardcoded):
  input_val: [4, 4096, 1024] f32, Wq/Wk/Wv: [64, 1024] f32, k_mask: [4, 4096] i32
  out = softmax(causal_mask(QK^T/sqrt(64))) @ V  -> [4, 4096, 64] f32

Sharding: 8 cores = 4 batches x 2 roles. Within a batch, queries are split
into 16 chunks of 256; role r takes chunks {2j+r}. Every core runs an
IDENTICAL program (SPMD); the chunk identity is carried entirely by the
per-core input data (xq rows, causal-mask tile contents).

Device program per core (all matmul operands in SBUF partitions 0:127,
contraction bases at partition 0 - base-64 operands crash this runtime):
  - K^T/V^T projection: lhsT = [Wk^T|Wv^T] (c-chunked), rhs = X^T -> PSUM
    [128=K^T(64)|V^T(64), 512] per t-block, copied to bf16 SBUF.
  - V^T -> V via DMA xbar transpose, packed as V1[kb] = [V | ones] (65 cols)
  - Q^T projection from xq (own queries, locally contiguous)
  - attention, flash style in S^T layout: for local chunk j (256 queries),
    k-block groups g=0..j (4 kb of 128 keys each):
      S^T psum [128, 4, 256] = K^T_kb.T @ Q^T_chunk  (K=64 contraction)
      P^T = exp(S^T / 8) via one ACT activation (N=1024)
      last group: P^T *= causal mask tile (DVE)
      O^T psum [65, 256] += V1[kb].T @ P^T  (fused numerator + denominator)
  - out [65, 2048] f32 = per-chunk O^T; host divides by row 64 and scatters.
"""
import numpy as np

B, T, C, H = 4, 4096, 1024, 64
N_CORES = 8
CC = 8          # contraction chunks of 128 over C
TB = 8          # key t-blocks of 512
QB = 4          # local query blocks of 512 (for Q projection)
NKB = 32        # key blocks of 128
NCH = 8         # local query chunks of 256
SCALE = 1.0 / np.sqrt(H)

_CACHE = {}


def _build_program(use_kmask: bool):
    from contextlib import ExitStack
    import concourse.tile as tile
    from concourse import bacc, mybir

    BF16 = mybir.dt.bfloat16
    F32 = mybir.dt.float32
    Exp = mybir.ActivationFunctionType.Exp

    nc = bacc.Bacc("TRN2", target_bir_lowering=False, debug=False,
                   num_devices=N_CORES)
    xt = nc.dram_tensor("xt", [TB, 128, CC, 512], BF16, kind="ExternalInput")
    xq = nc.dram_tensor("xq", [QB, 128, CC, 512], BF16, kind="ExternalInput")
    wkv = nc.dram_tensor("wkv", [128, CC, 128], BF16, kind="ExternalInput")
    wq = nc.dram_tensor("wq", [128, CC, 64], BF16, kind="ExternalInput")
    cm = nc.dram_tensor("cm", [128, 4, 256], BF16, kind="ExternalInput")
    if use_kmask:
        km = nc.dram_tensor("km", [128, NKB], BF16, kind="ExternalInput")
    o = nc.dram_tensor("o", [65, 2048], F32, kind="ExternalOutput")

    with tile.TileContext(nc) as tc:
        with ExitStack() as ctx:
            const = ctx.enter_context(tc.tile_pool(name="const", bufs=1))
            ppool = ctx.enter_context(tc.tile_pool(name="pt", bufs=3))
            projp = ctx.enter_context(tc.tile_pool(name="projp", bufs=1, space="PSUM"))
            stp = ctx.enter_context(tc.tile_pool(name="stp", bufs=3, space="PSUM"))
            otp = ctx.enter_context(tc.tile_pool(name="otp", bufs=1, space="PSUM"))

            XT = const.tile([128, TB, CC, 512], BF16)
            XQ = const.tile([128, QB, CC, 512], BF16)
            WKV = const.tile([128, CC, 128], BF16)
            WQ = const.tile([128, CC, 64], BF16)
            CM = const.tile([128, 4, 256], BF16)
            KT = const.tile([64, T], BF16)
            QT = const.tile([64, 2048], BF16)
            VT = const.tile([128, T], BF16)       # rows 64:128 hold V^T
            V1 = const.tile([128, NKB, 80], BF16)  # [:, kb, 0:64]=V, 64 = ones
            OUT = const.tile([65, 2048], F32)
            if use_kmask:
                KM = const.tile([128, NKB], BF16)
                nc.gpsimd.dma_start(KM[:], km.ap())

            nc.gpsimd.dma_start(WKV[:], wkv.ap())
            nc.gpsimd.dma_start(WQ[:], wq.ap())
            nc.gpsimd.dma_start(CM[:], cm.ap())
            nc.gpsimd.memset(V1[:, :, 64:65], 1.0)

            def emit_kv(tb):
                nc.gpsimd.dma_start(XT[:, tb], xt.ap()[tb])
                ps = projp.tile([128, 512], F32)
                for cc in range(CC):
                    nc.tensor.matmul(ps[:], WKV[:, cc, :], XT[:, tb, cc, :],
                                     start=(cc == 0), stop=(cc == CC - 1))
                sl = slice(512 * tb, 512 * (tb + 1))
                nc.vector.tensor_copy(KT[:, sl], ps[0:64, :])
                nc.vector.tensor_copy(VT[64:128, sl], ps[64:128, :])
                for kb in range(4 * tb, 4 * tb + 4):
                    nc.sync.dma_start_transpose(
                        V1[:, kb, 0:64], VT[64:128, 128 * kb:128 * (kb + 1)])

            def emit_q(qb):
                nc.gpsimd.dma_start(XQ[:, qb], xq.ap()[qb])
                ps = projp.tile([128, 512], F32)
                for cc in range(CC):
                    nc.tensor.matmul(ps[0:64, :], WQ[:, cc, :], XQ[:, qb, cc, :],
                                     start=(cc == 0), stop=(cc == CC - 1))
                nc.vector.tensor_copy(QT[:, 512 * qb:512 * (qb + 1)], ps[0:64, :])

            def emit_S(j, g):
                st = stp.tile([128, 4, 256], F32)
                for u in range(4):
                    kb = 4 * g + u
                    nc.tensor.matmul(st[:, u, :], KT[:, 128 * kb:128 * (kb + 1)],
                                     QT[:, 256 * j:256 * (j + 1)],
                                     start=True, stop=True)
                pt = ppool.tile([128, 4, 256], BF16)
                nc.scalar.activation(pt[:], st[:], Exp, scale=SCALE)
                if g == j:
                    nc.vector.tensor_mul(pt[:], pt[:], CM[:])
                if use_kmask:
                    for u in range(4):
                        kb = 4 * g + u
                        nc.vector.tensor_scalar_mul(pt[:, u, :], pt[:, u, :],
                                                    KM[:, kb:kb + 1])
                return pt

            def emit_O(j, g, pt, oacc):
                for u in range(4):
                    kb = 4 * g + u
                    nc.tensor.matmul(oacc[:], V1[:, kb, 0:65], pt[:, u, :],
                                     start=(kb == 0), stop=(kb == 4 * j + 3))
                if g == j:
                    nc.vector.tensor_copy(OUT[:, 256 * j:256 * (j + 1)], oacc[:])

            pending = None
            oaccs = {}
            for j in range(NCH):
                emit_kv(j)
                if j % 2 == 0:
                    emit_q(j // 2)
                oaccs[j] = otp.tile([65, 256], F32, name="oacc", tag="oacc")
                for g in range(j + 1):
                    pt = emit_S(j, g)
                    if pending is not None:
                        emit_O(pending[0], pending[1], pending[2], oaccs[pending[0]])
                    pending = (j, g, pt)
            emit_O(pending[0], pending[1], pending[2], oaccs[pending[0]])

            nc.sync.dma_start(o.ap(), OUT[:])

    nc.compile()
    return nc


def _get_program(use_kmask: bool):
    key = ("prog", use_kmask)
    if key not in _CACHE:
        _CACHE[key] = _build_program(use_kmask)
    return _CACHE[key]


def _host_prep(input_val, Wq, Wk, Wv, k_mask, use_kmask):
    import ml_dtypes
    bf = ml_dtypes.bfloat16

    wkv_np = np.empty((128, CC, 128), dtype=bf)
    wkv_np[:, :, 0:64] = Wk.reshape(64, CC, 128).transpose(2, 1, 0).astype(bf)
    wkv_np[:, :, 64:128] = Wv.reshape(64, CC, 128).transpose(2, 1, 0).astype(bf)
    wq_np = Wq.reshape(64, CC, 128).transpose(2, 1, 0).astype(bf).copy()

    kk = np.arange(128)[:, None]
    qq = np.arange(256)[None, :]
    cms = []
    for r in range(2):
        cm_np = np.empty((128, 4, 256), dtype=bf)
        for t in range(4):
            cm_np[:, t, :] = (kk <= 256 * r - 128 * t + qq).astype(bf)
        cms.append(cm_np)

    xts = []
    for b in range(B):
        Xb = np.asarray(input_val[b], dtype=np.float32)
        xts.append(np.ascontiguousarray(
            Xb.reshape(TB, 512, CC, 128).transpose(0, 3, 2, 1)).astype(bf))

    in_maps = []
    for c in range(N_CORES):
        b, r = c // 2, c % 2
        Xb = np.asarray(input_val[b], dtype=np.float32)
        rows = np.concatenate(
            [np.arange(256 * (2 * j + r), 256 * (2 * j + r) + 256)
             for j in range(NCH)])
        Xqrows = Xb[rows]  # [2048, 1024]
        xq_np = np.ascontiguousarray(
            Xqrows.reshape(QB, 512, CC, 128).transpose(0, 3, 2, 1)).astype(bf)
        m = {"xt": xts[b], "xq": xq_np, "wkv": wkv_np, "wq": wq_np,
             "cm": cms[r]}
        if use_kmask:
            m["km"] = np.asarray(k_mask[b], dtype=np.float32).reshape(
                NKB, 128).T.astype(bf).copy()
        in_maps.append(m)
    return in_maps


def _unshard(results):
    out = np.empty((B, T, H), dtype=np.float32)
    for c in range(N_CORES):
        b, r = c // 2, c % 2
        on = results[c]["o"]
        num = on[0:64, :]
        den = on[64, :]
        for j in range(NCH):
            g = 2 * j + r
            blk = num[:, 256 * j:256 * (j + 1)] / den[None, 256 * j:256 * (j + 1)]
            out[b, 256 * g:256 * (g + 1), :] = blk.T
    return out


def kernel(input_val, Wq, Wk, Wv, k_mask):
    import concourse.bass_utils as bu

    input_val = np.asarray(input_val)
    Wq, Wk, Wv = (np.asarray(a, dtype=np.float32) for a in (Wq, Wk, Wv))
    k_mask = np.asarray(k_mask)
    use_kmask = not bool(np.all(k_mask == 1))

    nc = _get_program(use_kmask)
    in_maps = _host_prep(input_val, Wq, Wk, Wv, k_mask, use_kmask)
    res = bu.run_bass_kernel_spmd(nc, in_maps, core_ids=list(range(N_CORES)))
    return _unshard(res.results)


def kernel_traced(input_val, Wq, Wk, Wv, k_mask, **trace_kwargs):
    """Like kernel() but returns (output, BassKernelResults) with tracing on.

    Used by test.py for HW timing; requires the antenv.axon_hooks shim (the
    caller sets it up)."""
    import concourse.bass_utils as bu

    input_val = np.asarray(input_val)
    k_mask = np.asarray(k_mask)
    use_kmask = not bool(np.all(k_mask == 1))
    nc = _get_program(use_kmask)
    in_maps = _host_prep(input_val, np.asarray(Wq, dtype=np.float32),
                         np.asarray(Wk, dtype=np.float32),
                         np.asarray(Wv, dtype=np.float32), k_mask, use_kmask)
    res = bu.run_bass_kernel_spmd(nc, in_maps, core_ids=list(range(N_CORES)),
                                  trace=True, **trace_kwargs)
    return _unshard(res.results), res


# revision 7
# speedup vs baseline: 1.1655x; 1.1474x over previous
"""Trainium2 Bass kernel for a single-head causal attention block.

Problem (hardcoded):
  input_val: [4, 4096, 1024] f32, Wq/Wk/Wv: [64, 1024] f32, k_mask: [4, 4096] i32
  out = softmax(causal_mask(QK^T/sqrt(64))) @ V  -> [4, 4096, 64] f32

Sharding: 8 cores = 4 batches x 2 roles. Within a batch, queries are split
into 16 chunks of 256; role r takes global chunks {2j+r}. Every core runs an
IDENTICAL program (SPMD); the chunk identity is carried entirely by the
per-core input data (xq row selection, causal-mask tile contents).

Device program per core (matmul contraction operands at partition base 0 -
base-64 operands crash this runtime):
  - K^T/V^T projection: lhsT = [Wk^T|Wv^T] (c-chunked), rhs = X^T -> PSUM
    [K^T rows 0:64 | V^T rows 64:128, 512] per t-block; DVE copies K^T to
    SBUF KT[0:64] and V^T (partition-shifted) to VT0[0:64], both bf16.
  - V1[kb] = [V | ones] built via PE-mode transpose of VT0 128-col blocks
    (bf16 PSUM out) + DVE copy; ones column memset once.
  - Q^T projection from xq (own queries, locally contiguous).
  - attention in S^T layout over local chunk-PAIRS m (512 queries), flash
    style; k-blocks (kb) of 128 keys, absolute range 0..8m+7:
      shared kbs 0..8m+3 (needed by both chunks): S^T [128,512] = K^T.T @ Q^T
        (K=64), batched per kb-pair into one [128,2,512] PSUM tile, one ACT
        exp (N=1024), causal mask (DVE) on the last two pairs, then per kb
        one O^T matmul [65,512] += V1[kb].T @ P^T (65th col = ones gives the
        softmax denominator for free).
      tail kbs 8m+4..8m+7 (second chunk only): same, N=256, accumulating
        into oacc[:, 256:512].
  - out [65, 2048] f32; host divides rows 0:64 by row 64 and scatters.
"""
import numpy as np

B, T, C, H = 4, 4096, 1024, 64
N_CORES = 8
CC = 8          # contraction chunks of 128 over C
TB = 8          # key t-blocks of 512
NKB = 32        # key blocks of 128
NPAIR = 4       # local chunk-pairs of 512 queries
SCALE = 1.0 / np.sqrt(H)

_CACHE = {}


def _build_program(use_kmask: bool):
    from contextlib import ExitStack
    import concourse.tile as tile
    from concourse import bacc, mybir

    BF16 = mybir.dt.bfloat16
    F32 = mybir.dt.float32
    Exp = mybir.ActivationFunctionType.Exp

    nc = bacc.Bacc("TRN2", target_bir_lowering=False, debug=False,
                   num_devices=N_CORES)
    xt = nc.dram_tensor("xt", [TB, 128, CC, 512], BF16, kind="ExternalInput")
    xq = nc.dram_tensor("xq", [NPAIR, 128, CC, 512], BF16, kind="ExternalInput")
    wkv = nc.dram_tensor("wkv", [128, CC, 128], BF16, kind="ExternalInput")
    wq = nc.dram_tensor("wq", [128, CC, 64], BF16, kind="ExternalInput")
    cm2 = nc.dram_tensor("cm2", [128, 4, 512], BF16, kind="ExternalInput")
    cm1 = nc.dram_tensor("cm1", [128, 2, 512], BF16, kind="ExternalInput")
    i64 = nc.dram_tensor("i64", [64, 64], BF16, kind="ExternalInput")
    if use_kmask:
        km = nc.dram_tensor("km", [128, NKB], BF16, kind="ExternalInput")
    o = nc.dram_tensor("o", [65, 2048], F32, kind="ExternalOutput")

    with tile.TileContext(nc) as tc:
        with ExitStack() as ctx:
            const = ctx.enter_context(tc.tile_pool(name="const", bufs=1))
            ppool = ctx.enter_context(tc.tile_pool(name="ptp", bufs=3))
            projp = ctx.enter_context(tc.tile_pool(name="projp", bufs=1, space="PSUM"))
            vtp = ctx.enter_context(tc.tile_pool(name="vtp", bufs=1, space="PSUM"))
            stp = ctx.enter_context(tc.tile_pool(name="stp", bufs=2, space="PSUM"))
            otp = ctx.enter_context(tc.tile_pool(name="otp", bufs=2, space="PSUM"))

            XT = const.tile([128, TB, CC, 512], BF16)
            XQ = const.tile([128, NPAIR, CC, 512], BF16)
            WKV = const.tile([128, CC, 128], BF16)
            WQ = const.tile([128, CC, 64], BF16)
            CM2 = const.tile([128, 4, 512], BF16)
            CM1 = const.tile([128, 2, 512], BF16)
            ID = const.tile([64, 64], BF16)
            KT = const.tile([64, T], BF16)
            QT = const.tile([64, 2048], BF16)
            VT0 = const.tile([64, T], BF16)
            V1 = const.tile([128, NKB, 80], BF16)  # [:, kb, 0:64]=V, col 64=1
            OUT = const.tile([65, 2048], F32)
            if use_kmask:
                KM = const.tile([128, NKB], BF16)
                nc.gpsimd.dma_start(KM[:], km.ap())

            nc.gpsimd.dma_start(WKV[:], wkv.ap())
            nc.gpsimd.dma_start(WQ[:], wq.ap())
            nc.gpsimd.dma_start(CM2[:], cm2.ap())
            nc.gpsimd.dma_start(CM1[:], cm1.ap())
            nc.gpsimd.dma_start(ID[:], i64.ap())
            nc.gpsimd.memset(V1[:, :, 64:65], 1.0)

            def emit_kv(tb):
                nc.gpsimd.dma_start(XT[:, tb], xt.ap()[tb])
                ps = projp.tile([128, 512], F32, name="ps", tag="ps")
                for cc in range(CC):
                    nc.tensor.matmul(ps[:], WKV[:, cc, :], XT[:, tb, cc, :],
                                     start=(cc == 0), stop=(cc == CC - 1))
                sl = slice(512 * tb, 512 * (tb + 1))
                nc.vector.tensor_copy(KT[:, sl], ps[0:64, :])
                nc.vector.tensor_copy(VT0[0:64, sl], ps[64:128, :])
                for kb in range(4 * tb, 4 * tb + 4):
                    vt_ps = vtp.tile([128, 64], BF16, name="vt_ps", tag="vt")
                    nc.tensor.transpose(vt_ps[:], VT0[:, 128 * kb:128 * (kb + 1)],
                                        ID[:])
                    nc.vector.tensor_copy(V1[:, kb, 0:64], vt_ps[:])

            def emit_q(m):
                nc.gpsimd.dma_start(XQ[:, m], xq.ap()[m])
                ps = projp.tile([128, 512], F32, name="ps", tag="ps")
                for cc in range(CC):
                    nc.tensor.matmul(ps[0:64, :], WQ[:, cc, :], XQ[:, m, cc, :],
                                     start=(cc == 0), stop=(cc == CC - 1))
                nc.vector.tensor_copy(QT[:, 512 * m:512 * (m + 1)], ps[0:64, :])

            def kmul(pt_slice, kb):
                nc.vector.tensor_scalar_mul(pt_slice, pt_slice, KM[:, kb:kb + 1])

            def emit_S_shared(m, sp):
                st = stp.tile([128, 2, 512], F32, name="st", tag="st")
                qsl = slice(512 * m, 512 * (m + 1))
                for u in range(2):
                    kb = 2 * sp + u
                    nc.tensor.matmul(st[:, u, :], KT[:, 128 * kb:128 * (kb + 1)],
                                     QT[:, qsl], start=True, stop=True)
                pt = ppool.tile([128, 2, 512], BF16, name="pt", tag="pt")
                nc.scalar.activation(pt[:], st[:], Exp, scale=SCALE)
                if sp >= 4 * m:
                    t0 = 2 * (sp - 4 * m)
                    nc.vector.tensor_mul(pt[:], pt[:], CM2[:, t0:t0 + 2, :])
                if use_kmask:
                    for u in range(2):
                        kmul(pt[:, u, :], 2 * sp + u)
                return pt

            def emit_O_shared(m, sp, pt, oacc):
                for u in range(2):
                    kb = 2 * sp + u
                    nc.tensor.matmul(oacc[:], V1[:, kb, 0:65], pt[:, u, :],
                                     start=(kb == 0), stop=False)

            def emit_S_tail(m):
                st = stp.tile([128, 2, 512], F32, name="st", tag="st")
                qsl = slice(512 * m + 256, 512 * (m + 1))
                for t in range(4):
                    kb = 8 * m + 4 + t
                    nc.tensor.matmul(
                        st[:, t // 2, 256 * (t % 2):256 * (t % 2) + 256],
                        KT[:, 128 * kb:128 * (kb + 1)],
                        QT[:, qsl], start=True, stop=True)
                pt = ppool.tile([128, 2, 512], BF16, name="pt", tag="pt")
                nc.scalar.activation(pt[:], st[:], Exp, scale=SCALE)
                nc.vector.tensor_mul(pt[:], pt[:], CM1[:])
                if use_kmask:
                    for t in range(4):
                        kmul(pt[:, t // 2, 256 * (t % 2):256 * (t % 2) + 256],
                             8 * m + 4 + t)
                return pt

            def emit_O_tail(m, pt, oacc):
                for t in range(4):
                    kb = 8 * m + 4 + t
                    nc.tensor.matmul(
                        oacc[:, 256:512], V1[:, kb, 0:65],
                        pt[:, t // 2, 256 * (t % 2):256 * (t % 2) + 256],
                        start=False, stop=(t == 3))
                nc.vector.tensor_copy(OUT[:, 512 * m:512 * (m + 1)], oacc[:])

            # skewed emission: the O-matmuls of work item i are emitted right
            # after the S-matmuls+exp of item i+1, keeping PE fed during exp.
            pending = None  # (kind, m, sp, pt)
            oaccs = {}

            def flush_pending():
                nonlocal pending
                if pending is None:
                    return
                kind, m_, sp_, pt_ = pending
                if kind == "shared":
                    emit_O_shared(m_, sp_, pt_, oaccs[m_])
                else:
                    emit_O_tail(m_, pt_, oaccs[m_])
                pending = None

            for m in range(NPAIR):
                emit_kv(2 * m)
                emit_kv(2 * m + 1)
                emit_q(m)
                oaccs[m] = otp.tile([65, 512], F32, name="oacc", tag="oacc")
                for sp in range(4 * m + 2):
                    pt = emit_S_shared(m, sp)
                    flush_pending()
                    pending = ("shared", m, sp, pt)
                pt = emit_S_tail(m)
                flush_pending()
                pending = ("tail", m, None, pt)
            flush_pending()

            nc.sync.dma_start(o.ap(), OUT[:])

    nc.compile()
    return nc


def _get_program(use_kmask: bool):
    key = ("prog", use_kmask)
    if key not in _CACHE:
        _CACHE[key] = _build_program(use_kmask)
    return _CACHE[key]


def _host_prep(input_val, Wq, Wk, Wv, k_mask, use_kmask):
    import ml_dtypes
    bf = ml_dtypes.bfloat16

    wkv_np = np.empty((128, CC, 128), dtype=bf)
    wkv_np[:, :, 0:64] = Wk.reshape(64, CC, 128).transpose(2, 1, 0).astype(bf)
    wkv_np[:, :, 64:128] = Wv.reshape(64, CC, 128).transpose(2, 1, 0).astype(bf)
    wq_np = Wq.reshape(64, CC, 128).transpose(2, 1, 0).astype(bf).copy()
    id_np = np.eye(64, dtype=np.float32).astype(bf)

    kk = np.arange(128)[:, None]
    qq = np.arange(256)[None, :]
    cm2s, cm1s = [], []
    for r in range(2):
        c1 = np.empty((128, 4, 256), dtype=bf)
        for t in range(4):
            c1[:, t, :] = (kk <= 256 * r - 128 * t + qq).astype(bf)
        cm1s.append(np.ascontiguousarray(c1).reshape(128, 2, 512).copy())
        c2 = np.ones((128, 4, 512), dtype=bf)
        c2[:, :, 0:256] = c1
        cm2s.append(c2)

    xts = []
    for b in range(B):
        Xb = np.asarray(input_val[b], dtype=np.float32)
        xts.append(np.ascontiguousarray(
            Xb.reshape(TB, 512, CC, 128).transpose(0, 3, 2, 1)).astype(bf))

    in_maps = []
    for c in range(N_CORES):
        b, r = c // 2, c % 2
        Xb = np.asarray(input_val[b], dtype=np.float32)
        rows = np.concatenate(
            [np.arange(256 * (2 * j + r), 256 * (2 * j + r) + 256)
             for j in range(2 * NPAIR)])
        Xqrows = Xb[rows]  # [2048, 1024]
        xq_np = np.ascontiguousarray(
            Xqrows.reshape(NPAIR, 512, CC, 128).transpose(0, 3, 2, 1)).astype(bf)
        m = {"xt": xts[b], "xq": xq_np, "wkv": wkv_np, "wq": wq_np,
             "cm2": cm2s[r], "cm1": cm1s[r], "i64": id_np}
        if use_kmask:
            m["km"] = np.asarray(k_mask[b], dtype=np.float32).reshape(
                NKB, 128).T.astype(bf).copy()
        in_maps.append(m)
    return in_maps


def _unshard(results):
    out = np.empty((B, T, H), dtype=np.float32)
    for c in range(N_CORES):
        b, r = c // 2, c % 2
        on = results[c]["o"]
        num = on[0:64, :]
        den = on[64, :]
        for j in range(2 * NPAIR):
            g = 2 * j + r
            blk = num[:, 256 * j:256 * (j + 1)] / den[None, 256 * j:256 * (j + 1)]
            out[b, 256 * g:256 * (g + 1), :] = blk.T
    return out


def kernel(input_val, Wq, Wk, Wv, k_mask):
    import concourse.bass_utils as bu

    input_val = np.asarray(input_val)
    Wq, Wk, Wv = (np.asarray(a, dtype=np.float32) for a in (Wq, Wk, Wv))
    k_mask = np.asarray(k_mask)
    use_kmask = not bool(np.all(k_mask == 1))

    nc = _get_program(use_kmask)
    in_maps = _host_prep(input_val, Wq, Wk, Wv, k_mask, use_kmask)
    res = bu.run_bass_kernel_spmd(nc, in_maps, core_ids=list(range(N_CORES)))
    return _unshard(res.results)


def kernel_traced(input_val, Wq, Wk, Wv, k_mask, **trace_kwargs):
    """Like kernel() but returns (output, BassKernelResults) with tracing on."""
    import concourse.bass_utils as bu

    input_val = np.asarray(input_val)
    k_mask = np.asarray(k_mask)
    use_kmask = not bool(np.all(k_mask == 1))
    nc = _get_program(use_kmask)
    in_maps = _host_prep(input_val, np.asarray(Wq, dtype=np.float32),
                         np.asarray(Wk, dtype=np.float32),
                         np.asarray(Wv, dtype=np.float32), k_mask, use_kmask)
    res = bu.run_bass_kernel_spmd(nc, in_maps, core_ids=list(range(N_CORES)),
                                  trace=True, **trace_kwargs)
    return _unshard(res.results), res
